# revision 1
# baseline (speedup 1.0000x reference)
"""Trainium2 Bass kernel for nn_EquivariantUpSampling_72773925864032.

Op (derived from the reference, verified numerically):
  inputs  x: (8, 128, 32, 32) f32,  p: (8, 3) int64 with entries in {0, 1}
  output  (8, 256, 64, 64) f32, zeros except, per sample i with
  (ph, pw, r) = p[i]:
      out[i, 2c + r, 2a + ph, 2b + pw] = rot_r(x[i, c])[a, b]
  where rot_0 = identity and rot_1[a, b] = x[b, (32 - a) % 32]
  (the reference's tile/rot90/crop trick reduces to these exact index maps;
  only r in {0, 1} is reachable since p = randint(0, 2)).

Strategy: pure data parallel, one sample per NeuronCore (8 cores).
Per-sample data-dependent choices (ph, pw, r) are folded into tiny
per-core *input values* so one SPMD program serves all cores:
  - one-hot weights select identity vs rot90 and the (ph, pw) quadrant
    (multiply-by-one-hot, the same trick the reference itself uses),
  - the channel offset r drives a register-dynamic destination offset
    (scalar_dynamic_offset DGE) on plain HWDGE output DMAs.
The 128 all-zero output channels are never written: run_bass_kernel_spmd /
bass2jax hand the NEFF zero-initialized output buffers by contract.

Schedule: raw bass (this toolchain allows ONE embedded sem wait per
instruction; standalone wait_ge + per-engine in-order completion
counters). Input is split across both HWDGE rings (SP + ACT); compute is
split DVE (feeder + even-row quads) / ACT (odd-row quads); output goes
out as two 1 MiB dynamic-offset DMAs (SP for half 0, ACT for half 1) so
the half-0 store overlaps half-1 compute.
"""

import numpy as np

B, C, H, W = 8, 128, 32, 32
OC, OH, OW = 2 * C, 2 * H, 2 * W
N_CORES = 8
NW = 8  # header words: w0..w5, r-bits, pad
XW_COLS = NW + H * W
HB = H // 2  # 16 input rows per half

_compiled = {}


def _build_bass():
    from contextlib import ExitStack

    import concourse.bass as bass
    import concourse.mybir as mybir

    fp32 = mybir.dt.float32
    # the ctor's trailing all-engine barrier only protects its const-AP
    # memsets, which this kernel never reads — skip it to start DMA earlier
    orig_aeb = bass.Bass.all_engine_barrier
    bass.Bass.all_engine_barrier = lambda self, **kw: None
    try:
        nc = bass.Bass(enable_partition_id=False)
    finally:
        bass.Bass.all_engine_barrier = orig_aeb

    Copy = mybir.ActivationFunctionType.Copy
    SP = mybir.EngineType.SP
    ACT = mybir.EngineType.Activation

    xw = nc.dram_tensor("xw", (C, XW_COLS), fp32, kind="ExternalInput")
    out = nc.dram_tensor("out", (OC, OH * OW), fp32, kind="ExternalOutput")

    ctx = ExitStack()
    with ctx:
        xin = ctx.enter_context(nc.sbuf_tensor([C, XW_COLS], fp32))
        tA = ctx.enter_context(nc.sbuf_tensor([C, H * W], fp32))
        tB = ctx.enter_context(nc.sbuf_tensor([C, H * W], fp32))
        t = ctx.enter_context(nc.sbuf_tensor([C, H * W], fp32))
        T = ctx.enter_context(nc.sbuf_tensor([C, OH * OW], fp32))

        s_a_in = nc.alloc_semaphore("s_a_in")  # header + x rows 0-15
        s_b_in = nc.alloc_semaphore("s_b_in")  # x rows 16-31
        s_v = nc.alloc_semaphore("s_v")  # DVE completion counter
        s_a = nc.alloc_semaphore("s_a")  # ACT completion counter
        s_out = nc.alloc_semaphore("s_out")

        xflat = xin[:, NW : NW + 1024]
        x3 = xflat.rearrange("p (a b) -> p a b", b=W)  # [c, row, col]
        wm = xin[:, 0:6]  # [m0, m1, w00, w01, w10, w11]
        rbits = xin[:, 6:7].bitcast(mybir.dt.int32)
        tB3 = tB[:].rearrange("p (a b) -> p a b", b=W)
        t3 = t[:].rearrange("p (a b) -> p a b", b=W)
        T3 = T[:].rearrange("p (a b) -> p a b", b=OW)
        # out rows (c two) -> [h][two][c][m]: channel 2c+two, half h
        vout = out[:].rearrange("(c two) (h m) -> h two c m", two=2, m=OH * OW // 2)

        # ---- split input load on both HWDGE rings ----
        cut = NW + HB * W
        nc.sync.dma_start(xin[:, 0:cut], xw[:, 0:cut]).then_inc(s_a_in, 16)
        nc.scalar.dma_start(xin[:, cut:XW_COLS], xw[:, cut:XW_COLS]).then_inc(
            s_b_in, 16
        )

        nc.sync.wait_ge(s_a_in, 16)
        nc.scalar.wait_ge(s_a_in, 16)
        rv = nc.values_load(
            rbits[0:1, 0:1],
            engines=[SP, ACT],
            min_val=0,
            max_val=1,
            skip_runtime_bounds_check=True,
        )

        # ---- DVE: feeder + even-row quads ----
        nc.vector.wait_ge(s_a_in, 16)
        nc.vector.tensor_scalar_mul(
            tA[:, 0 : HB * W], xflat[:, 0 : HB * W], wm[:, 0:1]
        ).then_inc(s_v, 1)  # v=1
        nc.vector.wait_ge(s_b_in, 16)
        # rot1[a, b] = x[b, (32-a) % 32], iterated source-naturally (reads
        # contiguous, scattered writes): half0 rows a in [0,16)
        nc.vector.tensor_scalar_mul(
            tB3[:, 0:1, :], x3[:, :, 0:1].transpose([0, 2, 1]), wm[:, 1:2]
        ).then_inc(s_v, 1)  # v=2
        nc.vector.tensor_scalar_mul(
            tB3[:, 15:0:-1, :].transpose([0, 2, 1]), x3[:, :, 17:32], wm[:, 1:2]
        ).then_inc(s_v, 1)  # v=3
        nc.vector.wait_ge(s_v, 3)  # same-engine RAW
        nc.vector.tensor_add(
            t[:, 0 : HB * W], tA[:, 0 : HB * W], tB[:, 0 : HB * W]
        ).then_inc(s_v, 1)  # v=4
        nc.vector.wait_ge(s_v, 4)
        for v in (0, 1):
            nc.vector.tensor_scalar_mul(
                T3[:, 0 : OH // 2 : 2, v::2], t3[:, 0:HB, :], wm[:, 2 + v : 3 + v]
            ).then_inc(s_v, 1)  # v=5,6
        # half 1
        nc.vector.tensor_scalar_mul(
            tB3[:, 31:15:-1, :].transpose([0, 2, 1]), x3[:, :, 1:17], wm[:, 1:2]
        ).then_inc(s_v, 1)  # v=7
        nc.vector.tensor_scalar_mul(
            tA[:, HB * W : H * W], xflat[:, HB * W : H * W], wm[:, 0:1]
        ).then_inc(s_v, 1)  # v=8
        nc.vector.wait_ge(s_v, 8)
        nc.vector.tensor_add(
            t[:, HB * W : H * W], tA[:, HB * W : H * W], tB[:, HB * W : H * W]
        ).then_inc(s_v, 1)  # v=9
        nc.vector.wait_ge(s_v, 9)
        for v in (0, 1):
            nc.vector.tensor_scalar_mul(
                T3[:, OH // 2 : OH : 2, v::2], t3[:, HB:H, :], wm[:, 2 + v : 3 + v]
            ).then_inc(s_v, 1)  # v=10,11

        # ---- ACT: odd-row quads ----
        nc.scalar.wait_ge(s_v, 4)
        for v in (0, 1):
            nc.scalar.activation(
                T3[:, 1 : OH // 2 : 2, v::2], t3[:, 0:HB, :], Copy,
                scale=wm[:, 4 + v : 5 + v],
            ).then_inc(s_a, 1)  # a=1,2
        nc.scalar.wait_ge(s_v, 9)
        for v in (0, 1):
            nc.scalar.activation(
                T3[:, OH // 2 + 1 : OH : 2, v::2], t3[:, HB:H, :], Copy,
                scale=wm[:, 4 + v : 5 + v],
            ).then_inc(s_a, 1)  # a=3,4

        # ---- output: two 1 MiB dynamic-offset HWDGE DMAs ----
        nc.sync.wait_ge(s_v, 6)
        nc.sync.wait_ge(s_a, 2)
        nc.sync.dma_start(
            vout[0][bass.ds(rv, 1)].squeeze(0), T[:, 0 : OH * OW // 2]
        ).then_inc(s_out, 16)
        nc.scalar.wait_ge(s_v, 11)
        nc.scalar.dma_start(
            vout[1][bass.ds(rv, 1)].squeeze(0), T[:, OH * OW // 2 : OH * OW]
        ).then_inc(s_out, 16)
        nc.sync.wait_ge(s_out, 32)
    return nc


def _get_bass():
    if "nc" not in _compiled:
        _compiled["nc"] = _build_bass()
    return _compiled["nc"]


def _make_in_maps(x, p):
    x = np.ascontiguousarray(np.asarray(x, dtype=np.float32))
    p = np.asarray(p)
    in_maps = []
    for i in range(B):
        ph, pw, r = int(p[i, 0]), int(p[i, 1]), int(p[i, 2])
        assert r in (0, 1) and ph in (0, 1) and pw in (0, 1)
        buf = np.zeros((C, XW_COLS), np.float32)
        w = np.zeros(NW, np.float32)
        w[0] = 1.0 if r == 0 else 0.0
        w[1] = 1.0 if r == 1 else 0.0
        for u in (0, 1):
            for v in (0, 1):
                w[2 + 2 * u + v] = 1.0 if (u == ph and v == pw) else 0.0
        w[6] = np.int32(r).view(np.float32)
        buf[:, 0:NW] = w
        buf[:, NW:] = x[i].reshape(C, H * W)
        in_maps.append({"xw": buf})
    return in_maps


def run(x, p, **spmd_kwargs):
    """Run the Bass kernel on 8 cores; returns (output, BassKernelResults)."""
    from concourse.bass_utils import run_bass_kernel_spmd

    nc = _get_bass()
    in_maps = _make_in_maps(x, p)
    res = run_bass_kernel_spmd(nc, in_maps, core_ids=list(range(N_CORES)), **spmd_kwargs)
    out = np.stack(
        [res.results[i]["out"].reshape(OC, OH, OW) for i in range(B)], axis=0
    )
    return out, res


def kernel(x, p):
    out, _ = run(x, p)
    return out

